# revision 24
# baseline (speedup 1.0000x reference)
"""Trainium2 Bass kernel for DiceLoss (hard-argmax dice, ignore background, mean).

Problem (hardcoded shapes):
  y_true: [16, 512, 512] int32 in [0, 8)
  y_pred: [16, 8, 512, 512] float32
  out   : scalar float32 = mean over classes 1..7 of
          (2*tp + eps) / (2*tp + fp + fn + eps)

Strategy v4 (8 NeuronCores, 2 images per core):
  ALL loads ride HWDGE. Earlier versions cast f32->bf16 in SWDGE
  DMAs, but SWDGE descriptor writing runs on the GpSimd Q7 cores and
  is ring-paced across the whole stream -- and DVE tensor_tensor ops
  (2x mode) use the shared SBUF port for src1, so Q7's descriptor
  writes and DVE's TT ops throttled EACH OTHER (~+300 cycles/op,
  stream stalls). HWDGE descriptor generation is RTL: zero Q7
  activity, zero port coupling. ScalarE (otherwise idle) does all
  f32/int32 -> bf16 conversion, trailing each chunk's arrival by ~2us.

  Work is split into 5 column sections -- img0 (0:1024), (1024:2048),
  img1 (0:1024), (1024:1536), (1536:2048) -- each loaded as 4
  channel-pair chunks. Per section: 4 pmax + 3 chain ops give the max,
  then pred masks. This spreads pred work across the stream (v2/v3's
  half-plane split piled 17us of preds after stream end). The LAST
  section's pmax ops read the f32 staging directly (TT 1x, bf16 out)
  so the tail max never waits on a convert; its preds use ScalarE
  conversions that complete in parallel.

  DVE op shapes: gt masks as 7 two-image ops (TS is_equal 4x) into ONE
  gt tile [P, 2, 7, NSUB, 129] (col 128 = ones); preds as
  channel-pair-merged TT is_equal against a stride-0-broadcast m.

  TensorE per class: psum bank [P, 387] = [G 0:258 | tp 258:387].
  G: lhsT=ones128, rhs=gt[:, both-images, c, s, :], 16 accums (rows
  identical = per-column gt sums), scheduled in PE's idle early window.
  tp/predcnt: lhsT=pred subtile, rhs=gt[n, c, s, 0:129], 32 accums
  (col 128 = pred counts).

  ScalarE: converts + G row-0 evacs (mid-stream) + tp evacs (tail).
  Host: tp = trace, pred_cnt = col-128 sums, gt_cnt = G row-0 sums.
"""

import numpy as np

EPS = 1e-05

N_CORES = 8
NB = 2
C = 8
P = 128
FD = 2048
NSUB = FD // 128

# (image, col offset, col length)
SECS = [(0, 0, 1024), (0, 1024, 1024), (1, 0, 1024), (1, 1024, 512), (1, 1536, 512)]

_CACHED_NC = None


def build_bass():
    from contextlib import ExitStack

    import concourse.bacc as bacc
    import concourse.tile as tile
    from concourse import mybir

    nc = bacc.Bacc(None, target_bir_lowering=False)

    yp = nc.dram_tensor("yp", [NB, C, P, FD], mybir.dt.float32, kind="ExternalInput")
    yt = nc.dram_tensor("yt", [NB, P, FD], mybir.dt.int32, kind="ExternalInput")
    mm_out = nc.dram_tensor("mm_out", [7, P, 129], mybir.dt.float32, kind="ExternalOutput")
    g_out = nc.dram_tensor("g_out", [7, 258], mybir.dt.float32, kind="ExternalOutput")

    # arrival stamps (ms): HWDGE starts ~8.5us, ~2.6us/MiB read;
    # SWDGE labels land ~9.8/12.4
    T_L = [0.0098, 0.0124]
    T_CHUNK = [
        [0.0163, 0.0189, 0.0215, 0.0241],   # img0 A
        [0.0267, 0.0293, 0.0319, 0.0345],   # img0 B
        [0.0371, 0.0397, 0.0423, 0.0449],   # img1 A
        [0.0462, 0.0475, 0.0488, 0.0501],   # img1 B1
        [0.0514, 0.0527, 0.0540, 0.0553],   # img1 B2
    ]
    CONV_LAG = 0.0021

    with tile.TileContext(nc) as tc, ExitStack() as ctx:
        chpool = ctx.enter_context(tc.tile_pool(name="ch", bufs=1))
        stgp = ctx.enter_context(tc.tile_pool(name="stg", bufs=4))
        mpool = ctx.enter_context(tc.tile_pool(name="mx", bufs=2))
        mtmp = ctx.enter_context(tc.tile_pool(name="mtmp", bufs=4))
        predp = ctx.enter_context(tc.tile_pool(name="pred", bufs=3))
        accp = ctx.enter_context(tc.tile_pool(name="acc", bufs=1))
        psump = ctx.enter_context(tc.tile_pool(name="psum", bufs=1, space="PSUM"))

        gtall = accp.tile([P, NB, 7, NSUB, 129], mybir.dt.bfloat16, name="gtall")
        ones128 = accp.tile([P, 128], mybir.dt.bfloat16, name="ones128")
        tfb = accp.tile([P, NB, FD], mybir.dt.bfloat16, name="tfb")
        psums = [
            psump.tile([P, 387], mybir.dt.float32, name=f"ps{c}", tag=f"ps{c}")
            for c in range(1, C)
        ]

        def g_ap(c):
            return psums[c - 1][:, 0:258]

        def tp_ap(c):
            return psums[c - 1][:, 258:387]

        nc.vector.memset(ones128, 1.0)
        nc.vector.memset(gtall[:, :, :, :, 128:129], 1.0)

        # ---- labels: SWDGE cast int32->bf16. Q7's descriptor work for
        # these 2 transfers finishes by ~9.5us, before DVE wakes up, so
        # no shared-port coupling; and ScalarE sheds two converts. ----
        for n in range(NB):
            nc.gpsimd.dma_start(out=tfb[:, n, :], in_=yt[n])

        # ---- yp chunks: HWDGE stage + ScalarE convert ----
        # chunks[sec][k] = bf16 tile [P, 2, ln] (channels 2k, 2k+1)
        chunks = []
        stgs = []
        for sec, (n, off, ln) in enumerate(SECS):
            row_c, row_s = [], []
            for k in range(4):
                st = stgp.tile([P, 2, ln], mybir.dt.float32, name="stg", tag="stg")
                nc.sync.dma_start(
                    out=st,
                    in_=yp[n, 2 * k : 2 * k + 2, :, off : off + ln].rearrange(
                        "c p x -> p c x"))
                bt = chpool.tile([P, 2, ln], mybir.dt.bfloat16,
                                 name=f"c{sec}_{k}", tag=f"c{sec}_{k}")
                with tc.tile_wait_until(T_CHUNK[sec][k] + CONV_LAG):
                    nc.scalar.copy(out=bt, in_=st)
                row_c.append(bt)
                row_s.append(st)
            chunks.append(row_c)
            stgs.append(row_s)

        def emit_gt(c, ts):
            tf4 = tfb[:].rearrange("p n (s f) -> p n s f", s=NSUB)
            with tc.tile_wait_until(ts):
                nc.vector.tensor_single_scalar(
                    out=gtall[:, :, c - 1, :, 0:128], in_=tf4,
                    scalar=float(c), op=mybir.AluOpType.is_equal,
                )

        def emit_gcnt(c, ts):
            for s in range(NSUB):
                with tc.tile_wait_until(ts):
                    nc.tensor.matmul(
                        psums[c - 1][0:1, 0:258], lhsT=ones128[:, 0:1],
                        rhs=gtall[:, :, c - 1, s, :],
                        start=(s == 0), stop=(s == NSUB - 1),
                    )

        def emit_tree(sec, ln, from_f32):
            """Non-tail sections: 4-op merged tree -- level1 maxes whole
            chunk tiles elementwise (pairing channels (0,2),(1,3) etc.,
            which a max doesn't care about), halving DVE op count.
            Tail section: per-chunk pmax + serial chain reading the f32
            staging directly, so only 2 small ops trail the last byte."""
            gates = T_CHUNK[sec] if from_f32 else [t + CONV_LAG + 0.001
                                                   for t in T_CHUNK[sec]]
            m = mpool.tile([P, 1024], mybir.dt.bfloat16, name="m", tag="m")
            if not from_f32:
                q01 = mtmp.tile([P, 2, 1024], mybir.dt.bfloat16, name="q01", tag="mt")
                with tc.tile_wait_until(gates[1]):
                    nc.vector.tensor_max(
                        q01[:, :, 0:ln], chunks[sec][0][:], chunks[sec][1][:])
                q23 = mtmp.tile([P, 2, 1024], mybir.dt.bfloat16, name="q23", tag="mt")
                with tc.tile_wait_until(gates[3]):
                    nc.vector.tensor_max(
                        q23[:, :, 0:ln], chunks[sec][2][:], chunks[sec][3][:])
                e = mtmp.tile([P, 2, 1024], mybir.dt.bfloat16, name="e", tag="mt")
                with tc.tile_wait_until(gates[3]):
                    nc.vector.tensor_max(e[:, :, 0:ln], q01[:, :, 0:ln], q23[:, :, 0:ln])
                    nc.vector.tensor_max(m[:, 0:ln], e[:, 0, 0:ln], e[:, 1, 0:ln])
                return m
            # serial f32 chain over staging chunk tiles: only two ops
            # (the last accumulate + the 2-channel fold) trail the final
            # byte, and nothing waits on a convert.
            acc = None
            for k in range(1, 4):
                t = mtmp.tile([P, 2, 512], mybir.dt.float32, name=f"a{k}", tag="mt")
                with tc.tile_wait_until(gates[k]):
                    nc.vector.tensor_max(
                        t[:], stgs[sec][k][:], acc if acc is not None
                        else stgs[sec][0][:])
                acc = t[:]
            with tc.tile_wait_until(gates[3]):
                nc.vector.tensor_max(m[:, 0:ln], acc[:, 0, :], acc[:, 1, :])
            return m

        def emit_preds_mm(sec, n, off, ln, m):
            """pred masks: c1 single, then channel-pair-merged ops with
            broadcast m; tp matmuls per class."""
            ns = ln // 128
            s0 = off // 128
            first, last = (sec == 0), (sec == len(SECS) - 1)
            ts = (T_CHUNK[sec][3] if sec == len(SECS) - 1
                  else T_CHUNK[sec][3] + CONV_LAG) + 0.0012
            mb = m[:, 0:ln].rearrange("p (o x) -> p o x", o=1).broadcast_to(
                [P, 2, ln])

            def mm(c, predv):
                for s in range(ns):
                    nc.tensor.matmul(
                        tp_ap(c),
                        lhsT=predv[:, s * 128 : (s + 1) * 128],
                        rhs=gtall[:, n, c - 1, s0 + s, :],
                        start=(first and s == 0),
                        stop=(last and s == ns - 1),
                    )

            p1 = predp.tile([P, 2, 1024], mybir.dt.bfloat16, name="p1", tag="pred")
            with tc.tile_wait_until(ts):
                nc.vector.tensor_tensor(
                    out=p1[:, 0, 0:ln], in0=chunks[sec][0][:, 1, :],
                    in1=m[:, 0:ln], op=mybir.AluOpType.is_equal)
            mm(1, p1[:, 0, 0:ln])
            for k in (1, 2, 3):
                pk = predp.tile([P, 2, 1024], mybir.dt.bfloat16, name=f"pk{k}", tag="pred")
                with tc.tile_wait_until(ts):
                    nc.vector.tensor_tensor(
                        out=pk[:, :, 0:ln], in0=chunks[sec][k],
                        in1=mb, op=mybir.AluOpType.is_equal)
                mm(2 * k, pk[:, 0, 0:ln])
                mm(2 * k + 1, pk[:, 1, 0:ln])

        # DVE program
        for c in range(1, C):
            emit_gt(c, T_L[1] + CONV_LAG)
        for c in range(1, C):
            emit_gcnt(c, T_L[1] + CONV_LAG + 0.008)

        # G evacs: right after each class's G accumulation stops (PE runs
        # G 16-30us), before ScalarE's convert queue gets tight
        evg = accp.tile([1, 7, 258], mybir.dt.float32, name="evg")
        for c in range(1, C):
            with tc.tile_wait_until(0.024 + 0.0012 * (c - 1)):
                nc.scalar.copy(out=evg[:, c - 1, :], in_=psums[c - 1][0:1, 0:258])
        nc.sync.dma_start(out=g_out[:], in_=evg)

        for sec, (n, off, ln) in enumerate(SECS):
            m = emit_tree(sec, ln, from_f32=(sec == len(SECS) - 1))
            emit_preds_mm(sec, n, off, ln, m)

        pt = accp.tile([P, 7, 129], mybir.dt.float32, name="pt")
        for c in range(1, C):
            nc.scalar.copy(out=pt[:, c - 1, :], in_=psums[c - 1][:, 258:387])
            nc.sync.dma_start(out=mm_out[c - 1], in_=pt[:, c - 1, :])

    nc.finalize()
    return nc


def _get_bass():
    global _CACHED_NC
    if _CACHED_NC is None:
        _CACHED_NC = build_bass()
    return _CACHED_NC


def make_in_maps(y_true, y_pred):
    yp = np.ascontiguousarray(np.asarray(y_pred, dtype=np.float32))
    yt = np.ascontiguousarray(np.asarray(y_true, dtype=np.int32))
    in_maps = []
    for i in range(N_CORES):
        yps = np.ascontiguousarray(yp[NB * i : NB * (i + 1)]).reshape(NB, C, P, FD)
        yts = np.ascontiguousarray(yt[NB * i : NB * (i + 1)]).reshape(NB, P, FD)
        in_maps.append({"yp": yps, "yt": yts})
    return in_maps


def epilogue(results):
    tp = np.zeros(7, dtype=np.float64)
    pred_cnt = np.zeros(7, dtype=np.float64)
    gt_cnt = np.zeros(7, dtype=np.float64)
    for r in results:
        mm = np.asarray(r["mm_out"], dtype=np.float64)  # [7, P, 129]
        tp += np.trace(mm[:, :, 0:128], axis1=1, axis2=2)
        pred_cnt += mm[:, :, 128].sum(axis=1)
        g = np.asarray(r["g_out"], dtype=np.float64)    # [7, 258]
        gt_cnt += g[:, 0:128].sum(axis=1) + g[:, 129:257].sum(axis=1)

    tp32 = tp.astype(np.float32)
    fp32_ = (pred_cnt - tp).astype(np.float32)
    fn32 = (gt_cnt - tp).astype(np.float32)
    eps = np.float32(EPS)
    two = np.float32(2.0)
    dice = (two * tp32 + eps) / (two * tp32 + fp32_ + fn32 + eps)
    return np.asarray(np.mean(dice, dtype=np.float32), dtype=np.float32)


def kernel(**inputs):
    from concourse.bass_utils import run_bass_kernel_spmd

    nc = _get_bass()
    in_maps = make_in_maps(inputs["y_true"], inputs["y_pred"])
    res = run_bass_kernel_spmd(nc, in_maps, core_ids=list(range(N_CORES)))
    return epilogue(res.results)


if __name__ == "__main__":
    rng = np.random.default_rng(0)
    y_true = rng.integers(0, C, size=(16, 512, 512)).astype(np.int32)
    y_pred = rng.standard_normal((16, C, 512, 512)).astype(np.float32)
    out = kernel(y_true=y_true, y_pred=y_pred)
    print("kernel output:", out)
